# revision 2
# baseline (speedup 1.0000x reference)
"""Depthwise 3D transposed conv (stride 2, k=4, SAME) on 8 trn2 NeuronCores.

x: (4, 32, 32, 32, 256) f32, filters: (4, 4, 4, 1, 256) f32
y: (4, 64, 64, 64, 256) f32

v4: J-plane-packed matmul geometry. Sharding: 8 cores = (batch n in 4) x
(h-halves in 2); each core computes all 64 output d-planes for a 32-row
h slab. Zero communication.

Per-dim polyphase (parity p): p=0: y[2m] = f3 x[m-1] + f1 x[m];
p=1: y[2m+1] = f2 x[m] + f0 x[m+1]. Every output has exactly 8 taps
(2 per dim).

Matmul packing: partitions = (t in 2, j in J, c in C) where t is a
w-shifted copy (+1 elem) of the input planes, j consecutive d-planes,
c channels. Columns = (rr in P, c' in C) with P = 2(J-1) output planes.
Each column accumulates 4 taps per matmul (2 kd x 2 kw); the 2 kh taps
are 2 matmuls (cls) accumulating in psum. With J=16, C=4: 480 useful
MACs/cycle vs 256 for the v3 plane-pair scheme - the PE does ALL 8 taps
in ~265us instead of 6/8 in ~405us.

Groups over the 64 d-planes: (29, 30, 5): g1 (k0=-1, cols 4:120 of the
30-slot pattern), g2 (k0=14, full 120), g3 (k0=29, J=4, C=16, 80 cols).
g1/g2 share the per-strip weight table (loaded once per strip); matmuls
for g1/g2 with the same stationary matrix are adjacent.

Evacuation (psum f32 -> f16 out slab) rotates Scalar/DVE; the t=1 shift
copy alternates DVE/Scalar; stores go via gpsimd SWDGE.
"""
import sys

sys.path.insert(0, "/opt/trn_rl_repo")

from contextlib import ExitStack

import numpy as np

import concourse.bass as bass  # noqa: F401  (registers engine classes)
import concourse.tile as tile
from concourse import bacc, mybir
from concourse.bass_utils import run_bass_kernel_spmd

F32 = mybir.dt.float32
F16 = mybir.dt.float16

N_CORES = 8
# per-dim taps: parity -> [(delta, k), ...]
TAPS = {0: [(-1, 3), (0, 1)], 1: [(0, 2), (1, 0)]}

# (name, k0, P, J, C, r0): outputs are global planes 2*k0+1+r0 .. +P-1
GROUPS = (("y1", -1, 29, 16, 4, 1),
          ("y2", 14, 30, 16, 4, 0),
          ("y3", 29, 5, 4, 16, 0))

_PROG = None


def _build_program():
    nc = bacc.Bacc(
        "TRN2", target_bir_lowering=False, debug=False, num_devices=N_CORES
    )
    # pre-tiled input: per strip, 128 partition rows (j, c, t); free dims
    # (g in 2, h, w) pack BOTH groups' plane windows so each strip needs one
    # fully-contiguous [128, 5184B] load. t=1 rows pre-shifted left 1 in w.
    xtp_d = nc.declare_dram_parameter("xtp", [64, 128, 2, 18, 36], F16, isOutput=False)
    xt3_d = nc.declare_dram_parameter("xt3", [16, 128, 18, 36], F16, isOutput=False)
    # weight tables: per strip, rows (t,j,c), 8 = (ph,pw,cls), cols (rr,c')
    wf_d = nc.declare_dram_parameter("wf", [64, 128, 8, 120], F16, isOutput=False)
    w3_d = nc.declare_dram_parameter("w3", [16, 128, 8, 80], F16, isOutput=False)
    y1_d = nc.declare_dram_parameter("y1", [64, 116, 2, 2, 16, 32], F16, isOutput=True)
    y2_d = nc.declare_dram_parameter("y2", [64, 120, 2, 2, 16, 32], F16, isOutput=True)
    y3_d = nc.declare_dram_parameter("y3", [16, 80, 2, 2, 16, 32], F16, isOutput=True)

    with ExitStack() as ctx:
        tc = ctx.enter_context(tile.TileContext(nc))
        wpool = ctx.enter_context(tc.tile_pool(name="wpool", bufs=4))
        xpool = ctx.enter_context(tc.tile_pool(name="xpool", bufs=8))
        opool = ctx.enter_context(tc.tile_pool(name="opool", bufs=6))
        ppool = ctx.enter_context(tc.tile_pool(name="ppool", bufs=4, space="PSUM"))

        ectr = [0]

        def evac(dst, src):
            # f32 psum -> f16 slab; GpSimd cannot read PSUM, so 1:1 S:V
            if ectr[0] % 2 == 0:
                nc.scalar.copy(dst, src)
            else:
                nc.vector.tensor_scalar_mul(dst, src, 1.0)
            ectr[0] += 1

        for s in range(64):
            wt = wpool.tile([128, 8, 120], F16, tag="w")
            nc.scalar.dma_start(out=wt[:], in_=wf_d[s])
            xt = xpool.tile([128, 2, 18, 36], F16, tag="xt")
            nc.sync.dma_start(out=xt[:], in_=xtp_d[s])
            ot1 = opool.tile([128, 2, 2, 16, 32], F16, tag="ot")
            ot2 = opool.tile([128, 2, 2, 16, 32], F16, tag="ot")
            for ph in range(2):
                pa = ppool.tile([128, 2, 16, 32], F32, tag="ps", name="ps")
                pb = ppool.tile([128, 2, 16, 32], F32, tag="ps", name="ps")
                for pw in range(2):
                    for cls in range(2):
                        dh = TAPS[ph][cls][0]
                        wap = wt[:, ph * 4 + pw * 2 + cls, :]
                        st, sp = (cls == 0), (cls == 1)
                        nc.tensor.matmul(
                            pa[0:120, pw], wap,
                            xt[:, 0, 1 + dh: 17 + dh, pw: pw + 32],
                            start=st, stop=sp,
                        )
                        nc.tensor.matmul(
                            pb[0:120, pw], wap,
                            xt[:, 1, 1 + dh: 17 + dh, pw: pw + 32],
                            start=st, stop=sp,
                        )
                # W cols ordered (r=1..29, r=0); g1 drops the trailing
                # r=0 block (plane -1), keeping psum base partition 0
                evac(ot1[0:116, ph], pa[0:116])
                evac(ot2[0:120, ph], pb[0:120])
            nc.gpsimd.dma_start(out=y1_d[s], in_=ot1[0:116])
            nc.gpsimd.dma_start(out=y2_d[s], in_=ot2[0:120])

        # g3: planes 59..63, J=4, C=16
        for s in range(16):
            wt = wpool.tile([128, 8, 80], F16, tag="w3")
            nc.scalar.dma_start(out=wt[:], in_=w3_d[s])
            xt = xpool.tile([128, 18, 36], F16, tag="xt3")
            nc.sync.dma_start(out=xt[:], in_=xt3_d[s])
            ot = opool.tile([128, 2, 2, 16, 32], F16, tag="ot")
            for ph in range(2):
                ps = ppool.tile([128, 2, 16, 32], F32, tag="ps", name="ps")
                for pw in range(2):
                    for cls in range(2):
                        dh = TAPS[ph][cls][0]
                        nc.tensor.matmul(
                            ps[0:80, pw],
                            wt[:, ph * 4 + pw * 2 + cls, :],
                            xt[:, 1 + dh: 17 + dh, pw: pw + 32],
                            start=(cls == 0), stop=(cls == 1),
                        )
                evac(ot[0:80, ph], ps[0:80])
            nc.gpsimd.dma_start(out=y3_d[s], in_=ot[0:80])
    nc.compile()
    return nc


def _get_program():
    global _PROG
    if _PROG is None:
        _PROG = _build_program()
    return _PROG


def _build_wtabs(ftap):
    """ftap: (kd, kh, kw, 256) f32 -> (wf [64,128,8,120], w3 [16,128,8,80])."""
    out = []
    for (J, C, P) in ((16, 4, 30), (4, 16, 5)):
        ns = 256 // C
        w = np.zeros((ns, 128, 8, P * C), np.float16)
        for rr in range(P):
            # J=16 table: col blocks ordered (r=1..29, r=0) so g1 can slice
            # a base-0 prefix; g3 table keeps natural order
            cb = (rr + P - 1) % P if J == 16 else rr
            jlo, jhi = rr // 2, rr // 2 + 1
            for j, kd in ((jlo, 2 if rr % 2 == 0 else 3),
                          (jhi, 0 if rr % 2 == 0 else 1)):
                if j >= J:
                    continue
                for t in range(2):
                    for ph in range(2):
                        for pw in range(2):
                            kw = TAPS[pw][t][1]
                            for cls in range(2):
                                kh = TAPS[ph][cls][1]
                                v = ftap[kd, kh, kw].reshape(ns, C)
                                q = ph * 4 + pw * 2 + cls
                                for c in range(C):
                                    # row order (j, c, t) matches the DMA's
                                    # partition fill from xin[j, c, t]
                                    w[:, (j * C + c) * 2 + t, q, cb * C + c] = \
                                        v[:, c]
        out.append(w)
    return out


def _make_in_maps(x, filters):
    x = np.ascontiguousarray(np.asarray(x), dtype=np.float32)
    filters = np.asarray(filters, dtype=np.float32)
    ftap = filters[:, :, :, 0, :]  # (kd, kh, kw, c)
    wf, w3 = _build_wtabs(ftap)

    in_maps = []
    for core in range(N_CORES):
        n, hb = core // 2, core % 2
        xin = np.zeros((34, 256, 2, 18, 36), np.float16)
        h0 = 16 * hb - 1
        hs, he = max(h0, 0), min(h0 + 18, 32)
        xin[1:33, :, 0, hs - h0:he - h0, 1:33] = \
            x[n, :, hs:he].transpose(0, 3, 1, 2)
        xin[:, :, 1, :, 0:35] = xin[:, :, 0, :, 1:36]

        def tiles(p0, J, C):
            ns = 256 // C
            a = xin[p0:p0 + J].reshape(J, ns, C, 2, 18, 36)
            return np.ascontiguousarray(
                a.transpose(1, 0, 2, 3, 4, 5)).reshape(ns, 128, 18, 36)

        xtp = np.stack([tiles(0, 16, 4), tiles(15, 16, 4)], axis=2)
        in_maps.append({"xtp": np.ascontiguousarray(xtp),
                        "xt3": tiles(30, 4, 16), "wf": wf, "w3": w3})
    return in_maps


def kernel(x, filters):
    nc = _get_program()
    in_maps = _make_in_maps(x, filters)
    res = run_bass_kernel_spmd(nc, in_maps, list(range(N_CORES)))
    y = np.empty((4, 64, 64, 64, 256), np.float32)
    for core in range(N_CORES):
        n, hb = core // 2, core % 2
        rc = res.results[core]
        yc = np.empty((64, 32, 64, 256), np.float32)
        for (name, k0, P, J, C, r0) in GROUPS:
            arr = rc[name].astype(np.float32)   # (S, P*C, 2, 2, 16, 32)
            S = arr.shape[0]
            arr = arr.reshape(S, P, C, 2, 2, 16, 32)
            # -> (P, a, ph, b, pw, S, C) -> (P, 32, 64, 256)
            arr = arr.transpose(1, 5, 3, 6, 4, 0, 2).reshape(P, 32, 64, 256)
            if J == 16:
                # full-pattern col block b maps to r = b+1 (b < 29) or r = 0;
                # g1 (P=29) keeps blocks 0..28 -> r = 1..29
                rs = ([b + 1 for b in range(29)] + [0])[:P]
                ls = np.array([2 * k0 + 1 + r for r in rs])
            else:
                ls = np.arange(2 * k0 + 1, 2 * k0 + 1 + P)
            yc[ls] = arr
        y[n, :, 32 * hb: 32 * hb + 32] = yc
    return y
